# revision 7
# baseline (speedup 1.0000x reference)
"""Trainium2 Bass kernel for DepthCueExtractor (v5).

out[b,h,w,f] = mean_{a,c}(lfi[b,a,h,w,c]) * hv[b,h,f]
where hv[b,w,f] = colmean_h(f_maps[b,h,w,f]) / max_w(colmean), evaluated at w=h.

Sharding: 8 cores = (batch b in 0..3) x (h-half j in 0..1). Each core gets
  - lfi[b, :, 128j:128j+128, :, :] host-transposed to [h, w, a, c]  (f32)
  - f_maps[b] rolled by -128j along w, CENTERED by -0.5, as fp8 e4m3
    (centering shrinks fp8 quantization error ~4x; the kernel adds back
    256*0.5=128 to the colsums, and hv = v/max(v) is scale-invariant so
    the /256 colmean normalization cancels)
and computes out[b, 128j:128j+128, :, :] (stored bf16, widened on host).

DMA-bound: 10.6 MB lfi + 4.2 MB fp8 fmap + 4.2 MB bf16 stores ~= 47 us at
the ~0.41 B/ns aggregate rate on one sync-queue FIFO; stores can only drain
after the loads, so the floor is ~start + 46.3us + teardown ~= 59 us. v5
packs compute into that shadow:
  - Loads: fmap f0..f3 first (hv chain gates every multiply), then lfi
    l0..l7 with tapered w-chunks [48,40,32,32,32,32,24,16] so the reduce
    tail shrinks; stores appended per chunk.
  - PE: fp8 DoubleRow colsum matmuls -> [1,2048] PSUM chunks; later a K=1
    ones(1/81) matmul broadcasts inv(max+128)/81 to 128 partitions.
  - Drains PSUM->hvrow alternate DVE (idle before l0 arrives) and ACT, so
    the chain paces with fmap arrivals instead of serializing on ACT.
  - GpSimd SWDGE: per-drain 32-descriptor mini-scatters hvrow -> hv0/hv1.
  - DVE: max-dance (max, +128 shift, 32x32 transposes, reciprocal), all lfi
    XY-reduces (f32, paced by DMA arrivals), hv0n=(hv0+128)*inv81 in bf16.
  - Multiplies run in DVE's 2x packed mode: m is duplicated into bf16 pairs
    md[h,w,2] so every operand of out[h,(w,f2,2)] = md*hv0n has a packed
    2-byte innermost dim (broadcast strides live on outer dims only).
    GpSimd takes chunks {0,1,3}, DVE the rest woven between reduces.

Precision: centered fp8 fmap ~3e-3; lfi/m stay f32 through the reduce (any
additive error in m fails near m~0 against the rel-err gate); m/hv0n/out
round to bf16 multiplicatively (~2^-9 each).
"""

import numpy as np
import ml_dtypes
from contextlib import ExitStack

import concourse.bass as bass
import concourse.bacc as bacc
import concourse.tile as tile
from concourse import mybir
from concourse.bass_utils import run_bass_kernel_spmd

F32 = mybir.dt.float32
BF16 = mybir.dt.bfloat16
FM_DT = mybir.dt.float8e4
FM_NP = ml_dtypes.float8_e4m3
B, A, H, W, C, F = 4, 9, 256, 256, 9, 64
AC = A * C
HL = H // 2  # 128 h rows per core
N_CORES = 8

_PROGRAM_CACHE = {}

WCHUNKS = [48, 40, 32, 32, 32, 32, 24, 16]  # lfi w-chunk taper, sum=256
WOFF = [sum(WCHUNKS[:i]) for i in range(len(WCHUNKS))]
NWC = len(WCHUNKS)
FCH = 4096
NFC = (W * F) // FCH  # 4 fmap DMA chunks
PCH = 2048  # PSUM colsum chunk

TT_GP = {0, 1, 3, 5}  # multiply chunks on GpSimd; rest on DVE
SHIFT = 128.0  # 256 rows * 0.5 centering offset, added back to colsums


def build_program() -> bass.Bass:
    nc = bacc.Bacc("TRN2", target_bir_lowering=False, debug=False)
    lfi = nc.declare_dram_parameter("lfi", [HL, W * AC], F32, isOutput=False)
    fmap = nc.declare_dram_parameter("fmap", [H, W * F], FM_DT, isOutput=False)
    ones2 = nc.declare_dram_parameter("ones2", [128, 32], FM_DT, isOutput=False)
    outp = nc.declare_dram_parameter("out", [HL, W * F], BF16, isOutput=True)

    with ExitStack() as ctx:
        tc = ctx.enter_context(tile.TileContext(nc))
        const_pool = ctx.enter_context(tc.tile_pool(name="const", bufs=1))
        fpool = ctx.enter_context(tc.tile_pool(name="fmap", bufs=3))
        ppool = ctx.enter_context(tc.tile_pool(name="psum", bufs=2, space="PSUM"))
        hvpool = ctx.enter_context(tc.tile_pool(name="hv", bufs=1))
        lpool = ctx.enter_context(tc.tile_pool(name="lfi", bufs=4))
        mpool = ctx.enter_context(tc.tile_pool(name="m", bufs=1))
        opool = ctx.enter_context(tc.tile_pool(name="outp", bufs=NWC))

        # ---- constants ----
        # ones2 viewed [128, 2, 16]: DoubleRow LDWEIGHTS needs the k-tile
        # stride to be a multiple of 16 bytes.
        ones2_t = const_pool.tile([128, 32], FM_DT)
        nc.sync.dma_start(out=ones2_t[:], in_=ones2[:])
        ones_col = const_pool.tile([1, 128], F32)
        nc.vector.memset(ones_col[:], 1.0 / AC)
        c_shift = const_pool.tile([128, 1], F32)
        nc.vector.memset(c_shift[:], SHIFT)
        c_zero = const_pool.tile([128, 1], F32)
        nc.vector.memset(c_zero[:], 0.0)

        # ---- loads: ALL on the sync queue in one deterministic order ----
        # fmap first: the hv chain gates every multiply; lfi then streams
        # back-to-back so the DVE reduce pipeline is paced only by DMA.
        lfi_w = lfi.rearrange("p (w a c) -> p w a c", a=A, c=C)
        fmap_h = fmap.rearrange("(hh p) c -> p hh c", hh=2)  # [128, 2, W*F]
        lts = [None] * NWC
        fts = [None] * NFC

        for fc in range(NFC):
            ft = fpool.tile([128, 2, FCH], FM_DT, tag="ft", name=f"ft{fc}")
            nc.sync.dma_start(
                out=ft[:], in_=fmap_h[:, :, FCH * fc : FCH * (fc + 1)]
            )
            fts[fc] = ft
        for wc in range(NWC):
            lt = lpool.tile([128, WCHUNKS[wc], A, C], F32, tag="lt", name=f"lt{wc}")
            nc.sync.dma_start(
                out=lt[:], in_=lfi_w[:, WOFF[wc] : WOFF[wc] + WCHUNKS[wc], :, :]
            )
            lts[wc] = lt

        # ---- PE colsums (fp8 DoubleRow) + drains (DVE/ACT) + scatters ----
        hvrow = hvpool.tile([1, W * F], F32, tag="hvrow")
        ones_dr = ones2_t.rearrange("p (k s) -> p k s", k=2)[:, :, 0:1]  # [128,2,1]
        for pc in range((W * F) // PCH):  # 8 psum chunks
            ft = fts[pc // 2]
            base = PCH * (pc % 2)
            cs = ppool.tile([1, PCH], F32, tag="cs")
            for s in range(PCH // 512):
                nc.tensor.matmul(
                    cs[:, 512 * s : 512 * (s + 1)],
                    ones_dr,
                    ft[:, :, base + 512 * s : base + 512 * (s + 1)],
                    start=True,
                    stop=True,
                    perf_mode=mybir.MatmulPerfMode.DoubleRow,
                )
            # drain PSUM -> hvrow; DVE is idle until l0 lands (~24us), so
            # alternate DVE/ACT to pace with the fmap chunk arrivals.
            dsl = hvrow[:, PCH * pc : PCH * (pc + 1)]
            if pc % 2 == 0:
                nc.vector.tensor_tensor(
                    out=dsl,
                    in0=cs[:],
                    in1=c_zero[0:1, 0:1].broadcast_to([1, PCH]),
                    op=mybir.AluOpType.add,  # +0: copy
                )
            else:
                nc.scalar.copy(dsl, cs[:])
            # scatter this 32-w slice to hv0/hv1 right away (32 descriptors)
            if pc == 0:
                hv0 = hvpool.tile([128, F], F32, tag="hv0")
            if pc == 4:
                hv1 = hvpool.tile([128, F], F32, tag="hv1")
            dst = hv0 if pc < 4 else hv1
            nc.gpsimd.dma_start(
                out=dst[32 * (pc % 4) : 32 * (pc % 4) + 32, :],
                in_=dsl.rearrange("p (w f) -> p w f", w=32),
            )

        # ---- max over 256 w via 32x32 transposes (DVE), then inv bcast ----
        hm = hvpool.tile([128, F], F32, tag="hm")
        nc.vector.tensor_max(hm[:], hv0[:], hv1[:])
        hmT = hvpool.tile([F, 128], F32, tag="hmT")
        for pi in range(4):
            for fj in range(F // 32):
                nc.vector.transpose(
                    out=hmT[32 * fj : 32 * (fj + 1), 32 * pi : 32 * (pi + 1)],
                    in_=hm[32 * pi : 32 * (pi + 1), 32 * fj : 32 * (fj + 1)],
                )
        mxc = hvpool.tile([F, 32], F32, tag="mxc")
        nc.vector.memset(mxc[:], 0.0)
        nc.vector.reduce_max(out=mxc[:, 0:1], in_=hmT[:], axis=mybir.AxisListType.X)
        mxr = hvpool.tile([32, F], F32, tag="mxr")
        for pi in range(F // 32):
            nc.vector.transpose(
                out=mxr[0:32, 32 * pi : 32 * (pi + 1)],
                in_=mxc[32 * pi : 32 * (pi + 1), 0:32],
            )
        # max(colmean)*256 = max(colsum'+128): add the centering shift back.
        inv_in = hvpool.tile([1, F], F32, tag="inv_in")
        nc.vector.tensor_tensor(
            out=inv_in[:],
            in0=mxr[0:1, :],
            in1=c_shift[0:1, 0:1].broadcast_to([1, F]),
            op=mybir.AluOpType.add,
        )
        inv_row = hvpool.tile([1, F], F32, tag="inv_row")
        nc.vector.reciprocal(inv_row[:], inv_in[:])

        inv_rep = ppool.tile([128, F], F32, tag="cs")
        nc.tensor.matmul(inv_rep[:], ones_col[:], inv_row[:], start=True, stop=True)
        inv81 = hvpool.tile([128, F], F32, tag="inv81")
        nc.scalar.copy(inv81[:], inv_rep[:])

        hv0s = hvpool.tile([128, F], F32, tag="hv0s")
        nc.vector.tensor_tensor(
            out=hv0s[:],
            in0=hv0[:],
            in1=c_shift[:, 0:1].broadcast_to([128, F]),
            op=mybir.AluOpType.add,
        )
        hv0n = hvpool.tile([128, F], BF16, tag="hv0n")
        nc.vector.tensor_tensor(
            out=hv0n[:], in0=hv0s[:], in1=inv81[:], op=mybir.AluOpType.mult
        )
        hv0n_p = hv0n.rearrange("p (fo fi) -> p fo fi", fi=2)  # [128, 32, 2]

        # ---- per-chunk: reduce (DVE) -> bf16 pair-dup -> multiply -> store
        # Multiply operands all have packed 2-byte innermost dims, enabling
        # DVE's 2x mode: out[p, w, fo, fi] = md[p, w, *, fi] * hv0n[p, *, fo, fi].
        m = mpool.tile([128, W], F32, tag="m")
        md = mpool.tile([128, W, 2], BF16, tag="md")
        for wc in range(NWC):
            w0, wn = WOFF[wc], WCHUNKS[wc]
            sl = slice(w0, w0 + wn)
            nc.vector.reduce_sum(
                out=m[:, sl], in_=lts[wc][:], axis=mybir.AxisListType.XY
            )
            eng = nc.gpsimd if wc in TT_GP else nc.vector
            # duplicate m into bf16 pairs on the engine that multiplies
            eng.tensor_tensor(
                out=md[:, sl, :],
                in0=m[:, sl].unsqueeze(2).broadcast_to([128, wn, 2]),
                in1=c_zero[:, 0:1].unsqueeze(2).broadcast_to([128, wn, 2]),
                op=mybir.AluOpType.add,  # +0: cast to bf16
            )
            out_t = opool.tile([128, wn, 32, 2], BF16, tag="ot", name=f"ot{wc}")
            eng.tensor_tensor(
                out=out_t[:],
                in0=md[:, sl, :].unsqueeze(2).broadcast_to([128, wn, 32, 2]),
                in1=hv0n_p.unsqueeze(1).broadcast_to([128, wn, 32, 2]),
                op=mybir.AluOpType.mult,
            )
            nc.sync.dma_start(
                out=outp[:, F * w0 : F * (w0 + wn)],
                in_=out_t[:].rearrange("p w fo fi -> p (w fo fi)"),
            )

    nc.compile()
    return nc


def _get_program() -> bass.Bass:
    if "nc" not in _PROGRAM_CACHE:
        _PROGRAM_CACHE["nc"] = build_program()
    return _PROGRAM_CACHE["nc"]


def make_in_maps(lfi: np.ndarray, f_maps: np.ndarray) -> list[dict]:
    in_maps = []
    for core in range(N_CORES):
        b, j = divmod(core, 2)
        lfi_s = np.ascontiguousarray(
            lfi[b, :, HL * j : HL * (j + 1), :, :].transpose(1, 2, 0, 3)
        ).reshape(HL, W * AC)
        fm = (np.roll(f_maps[b], -HL * j, axis=1) - 0.5).reshape(H, W * F)
        in_maps.append(
            {
                "lfi": lfi_s,
                "fmap": np.ascontiguousarray(fm.astype(FM_NP)),
                "ones2": np.ones((128, 32), FM_NP),
            }
        )
    return in_maps


def assemble_out(results: list[dict]) -> np.ndarray:
    out = np.empty((B, H, W, F), np.float32)
    for core in range(N_CORES):
        b, j = divmod(core, 2)
        out[b, HL * j : HL * (j + 1)] = (
            results[core]["out"].astype(np.float32).reshape(HL, W, F)
        )
    return out


def kernel(lfi: np.ndarray, f_maps: np.ndarray) -> np.ndarray:
    lfi = np.asarray(lfi, dtype=np.float32)
    f_maps = np.asarray(f_maps, dtype=np.float32)
    nc = _get_program()
    in_maps = make_in_maps(lfi, f_maps)
    res = run_bass_kernel_spmd(nc, in_maps, list(range(N_CORES))).results
    return assemble_out(res)
